# revision 35
# baseline (speedup 1.0000x reference)
"""TRN2 Bass kernel for nn_Aligner (dense_transformer).

Reference computation (per batch b):
    ex  = ix_b @ W.T + b         [L, D]
    eo  = io_b @ W.T + b         [L, D]
    s   = ex @ eo.T              [L, L]
    a   = softmax(s, axis=-1)
    out = a @ io_b               [L, D]

Device algorithm (algebraic restructure; softmax is shift-invariant):
    s[l, m] + c[m] = ix_l @ M @ io_m + c[m]   (row-const dropped in softmax)
      where M = W.T @ W, c = io @ (W.T b), both host-precomputed.
    tT[e, l] = sum_d M[d, e] ixT[d, l]                   (step A)
    sb[l, m] = sum_e tT[e, l] ioT[e, m]                  (step B, l-major)
    E[l, m]  = exp(sb - rowmax(sb))  (fp8; rowmax via DVE top-8)
    ET       = E transposed to [m, l] (PE fp8 transposes)
    sums[l]  = sum_m ET[m, l] wc[m],  wc = exp(c)        (tiny DR matmuls)
    out[l,d] = (sum_m ET[m, l] wc[m] io[m, d]) / sums[l] (step C)
    The e^c weights fold softmax's c-bias into host-prepped operands
    (iw = wc*io), so no per-m bias is ever needed on device.

All three big matmuls run as fp8e4 DoubleRow (cost-model 0.5 cyc/row at
K=256/instr) with hi/lo compensation: X*Y ~= Xh@Yh + Xl@Yh + Xh@Yl where
Xh = fp8(S*X), Xl = fp8(S*X - Xh), pricing A/B at 0.75x and C at 0.5x of
their fp16 cost while keeping ~bf16-level accuracy (end-to-end rel err
~9e-3 vs the 2e-2 gate). Power-of-2 scales keep operands out of fp8's
subnormal range; inverse scales fold into the exp activation's scale.
M/ix/io splits are host-precomputed; the t split is made on device
(Act fp8 cast + one DVE scalar_tensor_tensor per chunk).

PE issue order per batch: A(b) | B(b).lc01 | T(b-1) | B(b).lc23 |
sums+C(b-1). The mid-B placement of the previous batch's E-transposes
lets their Act/DVE copies drain before this batch's exps/max need those
queues; A(b)'s t-cast trail hides under B's first lc groups. Cold-start
DMAs issue as kp-pair transfers spread over the SP/Act/Pool sequencers
(each dma_start costs ~0.6us of one sequencer), and the final stores are
quarter-split across engines to shorten the last DMA chain.

Sharding: pure data-parallel over batch. 32 batches / 8 cores = 4 per
core. M, identity, ones are replicated; no collectives.
"""
import sys

if "/opt/trn_rl_repo" not in sys.path:
    sys.path.insert(0, "/opt/trn_rl_repo")

import numpy as np
import ml_dtypes
from contextlib import ExitStack

import concourse.bacc as bacc
import concourse.mybir as mybir
from concourse import tile

B, L, D = 32, 512, 1024
NCORES = 8
BL = B // NCORES          # batches per core
P = 128
DC = D // P               # 8 contraction chunks over d/e
KP = DC // 2              # 4 DoubleRow k-tile pairs
LC = L // P               # 4 chunks over l
MC = L // P               # 4 chunks over m
HD = D // 2               # 512: fp32 PSUM free-dim limit per matmul
F32 = mybir.dt.float32
F16 = mybir.dt.float16
BF16 = mybir.dt.bfloat16
F8 = mybir.dt.float8e4
NF8 = ml_dtypes.float8_e4m3
NBF16 = ml_dtypes.bfloat16
EXP = mybir.ActivationFunctionType.Exp
COPY = mybir.ActivationFunctionType.Copy
DR = mybir.MatmulPerfMode.DoubleRow
MUL = mybir.AluOpType.mult
SUB = mybir.AluOpType.subtract

SM = 64.0                 # M fp8 scale
SX = 16.0                 # ix fp8 scale
SO = 16.0                 # io fp8 scale (step B operand)
ST = 16.0                 # t fp8 scale (device-side split)
A_CAST = ST / (SM * SX)   # PSUM(A) -> fp8 t scale: 1/64
B_EXP = 1.0 / (SO * ST)   # PSUM(B) -> exp scale: 1/256

_CACHE = {}


def _build_program():
    nc = bacc.Bacc("TRN2", target_bir_lowering=False, debug=False,
                   num_devices=NCORES)
    xh_d = nc.dram_tensor("xh", [BL, D, L], F8, kind="ExternalInput")
    xl_d = nc.dram_tensor("xl", [BL, D, L], F8, kind="ExternalInput")
    oh_d = nc.dram_tensor("oh", [BL, D, L], F8, kind="ExternalInput")
    ol_d = nc.dram_tensor("ol", [BL, D, L], F8, kind="ExternalInput")
    ih_d = nc.dram_tensor("ih", [BL, L, D], F8, kind="ExternalInput")
    il_d = nc.dram_tensor("il", [BL, L, D], F8, kind="ExternalInput")
    mh_d = nc.dram_tensor("mh", [D, D], F8, kind="ExternalInput")
    ml_d = nc.dram_tensor("ml", [D, D], F8, kind="ExternalInput")
    wh_d = nc.dram_tensor("wh", [BL, L], F8, kind="ExternalInput")
    wl_d = nc.dram_tensor("wl", [BL, L], F8, kind="ExternalInput")
    out_d = nc.dram_tensor("out", [BL, L, D], F16, kind="ExternalOutput")

    ones_dram = nc.inline_tensor(np.ones((P, 2), dtype=NBF16), name="ones_c")
    ident_dram = nc.inline_tensor(np.eye(P, dtype=NF8), name="ident_c")

    with tile.TileContext(nc) as tc, ExitStack() as ctx:
        const = ctx.enter_context(tc.tile_pool(name="const", bufs=1))
        xpool = ctx.enter_context(tc.tile_pool(name="xp", bufs=2))
        opool = ctx.enter_context(tc.tile_pool(name="op", bufs=2))
        ipool = ctx.enter_context(tc.tile_pool(name="ip", bufs=2))
        tpool = ctx.enter_context(tc.tile_pool(name="tp", bufs=2))
        epool = ctx.enter_context(tc.tile_pool(name="ep", bufs=2))
        outp = ctx.enter_context(tc.tile_pool(name="out", bufs=4))
        small = ctx.enter_context(tc.tile_pool(name="small", bufs=2))

        mm_psum = ctx.enter_context(tc.tile_pool(name="mmp", bufs=4, space="PSUM"))
        c_psum = ctx.enter_context(tc.tile_pool(name="cp", bufs=4, space="PSUM"))

        # ---- constants + PE/exp warm-up (rides the cold DMA window) ----
        ones2 = const.tile([P, 2], BF16)
        nc.sync.dma_start(ones2[:], ones_dram.ap())
        warm = small.tile([P, 2], F32, tag="warm")
        nc.scalar.activation(warm[:], ones2[:], EXP)
        warm_ps = mm_psum.tile([P, 2], F32, tag="mm", name="warmps")
        for i in range(60):
            nc.tensor.matmul(warm_ps[:2, :], ones2[:], ones2[:],
                             start=(i == 0), stop=(i == 59))

        mh_sb = const.tile([P, DC * D], F8)
        ml_sb = const.tile([P, DC * D], F8)
        mh3 = mh_sb[:].rearrange("p (dc e) -> p dc e", dc=DC)
        ml3 = ml_sb[:].rearrange("p (dc e) -> p dc e", dc=DC)
        ident = const.tile([P, P], F8)

        def load_x_cold(b):
            # batch 0: interleave x and M chunks so DR round kp needs only
            # the first 2(kp+1) chunks of each
            xh = xpool.tile([P, DC * L], F8, tag="xh")
            xl = xpool.tile([P, DC * L], F8, tag="xl")
            xh3l = xh[:].rearrange("p (dc l) -> p dc l", dc=DC)
            xl3l = xl[:].rearrange("p (dc l) -> p dc l", dc=DC)
            mh3l = mh_sb[:].rearrange("p (dc e) -> p dc e", dc=DC)
            ml3l = ml_sb[:].rearrange("p (dc e) -> p dc e", dc=DC)
            # One DMA per kp-pair (each dma_start costs ~0.6us of its issue
            # queue), spread across three sequencers: SP paces the pass-1
            # operands (xh+mh), Pool ml (pass 2), Act xl (pass 3).
            for kp in range(KP):
                s = slice(2 * kp, 2 * kp + 2)
                rows = slice(2 * kp * P, (2 * kp + 2) * P)
                nc.sync.dma_start(
                    xh3l[:, s, :],
                    xh_d[b, rows, :].rearrange("(dc p) l -> p dc l", p=P))
                nc.sync.dma_start(
                    mh3l[:, s, :],
                    mh_d[rows, :].rearrange("(dc p) e -> p dc e", p=P))
                nc.gpsimd.dma_start(
                    ml3l[:, s, :],
                    ml_d[rows, :].rearrange("(dc p) e -> p dc e", p=P))
                nc.scalar.dma_start(
                    xl3l[:, s, :],
                    xl_d[b, rows, :].rearrange("(dc p) l -> p dc l", p=P))
            nc.scalar.dma_start(ident[:], ident_dram.ap())
            return xh, xl

        def load_x(b):
            xh = xpool.tile([P, DC * L], F8, tag="xh")
            xl = xpool.tile([P, DC * L], F8, tag="xl")
            nc.sync.dma_start(xh[:].rearrange("p (dc l) -> p dc l", dc=DC),
                              xh_d[b].rearrange("(dc p) l -> p dc l", p=P))
            nc.sync.dma_start(xl[:].rearrange("p (dc l) -> p dc l", dc=DC),
                              xl_d[b].rearrange("(dc p) l -> p dc l", p=P))
            return xh, xl

        def load_rest(b):
            oh = opool.tile([P, DC * L], F8, tag="oh")
            ol = opool.tile([P, DC * L], F8, tag="ol")
            nc.sync.dma_start(oh[:].rearrange("p (et m) -> p et m", et=DC),
                              oh_d[b].rearrange("(et p) m -> p et m", p=P))
            nc.sync.dma_start(ol[:].rearrange("p (et m) -> p et m", et=DC),
                              ol_d[b].rearrange("(et p) m -> p et m", p=P))
            ih = ipool.tile([P, MC * D], F8, tag="ih")
            il = ipool.tile([P, MC * D], F8, tag="il")
            nc.sync.dma_start(ih[:].rearrange("p (mt d) -> p mt d", mt=MC),
                              ih_d[b].rearrange("(mt p) d -> p mt d", p=P))
            nc.sync.dma_start(il[:].rearrange("p (mt d) -> p mt d", mt=MC),
                              il_d[b].rearrange("(mt p) d -> p mt d", p=P))
            wch = small.tile([P, MC], F8, tag="wch")
            wcl = small.tile([P, MC], F8, tag="wcl")
            nc.gpsimd.dma_start(wch[:], wh_d[b].rearrange("(mc p) -> p mc", p=P))
            nc.gpsimd.dma_start(wcl[:], wl_d[b].rearrange("(mc p) -> p mc", p=P))
            return oh, ol, ih, il, wch, wcl

        def a_psum_tiles():
            # c-pool banks first: they free earliest in the preceding TC
            # (wave-1 muls), and A's early ec groups need banks immediately
            pas = [c_psum.tile([P, L], F32, tag="pc", name=f"pa{j}")
                   for j in range(4)]
            pas += [mm_psum.tile([P, L], F32, tag="mm", name=f"pa{j}")
                    for j in range(4, DC)]
            return pas

        def a_drs(pa, ec, kp, xh3, xl3, first, last):
            lh = mh3[:, 2 * kp:2 * kp + 2, ec * P:(ec + 1) * P]
            ll = ml3[:, 2 * kp:2 * kp + 2, ec * P:(ec + 1) * P]
            rh = xh3[:, 2 * kp:2 * kp + 2, :]
            rl = xl3[:, 2 * kp:2 * kp + 2, :]
            nc.tensor.matmul(pa[:], lh, rh, start=first, stop=False, perf_mode=DR)
            nc.tensor.matmul(pa[:], ll, rh, start=False, stop=False, perf_mode=DR)
            nc.tensor.matmul(pa[:], lh, rl, start=False, stop=last, perf_mode=DR)

        def a_cast(pa, ec, th8, tl8):
            nc.scalar.activation(th8[:, ec * L:(ec + 1) * L], pa[:], COPY,
                                 scale=A_CAST)
            nc.vector.scalar_tensor_tensor(
                tl8[:, ec * L:(ec + 1) * L], pa[:], A_CAST,
                th8[:, ec * L:(ec + 1) * L], MUL, SUB)

        def step_a(xh, xl, cold):
            th8 = tpool.tile([P, DC * L], F8, tag="th")
            tl8 = tpool.tile([P, DC * L], F8, tag="tl")
            xh3 = xh[:].rearrange("p (dc l) -> p dc l", dc=DC)
            xl3 = xl[:].rearrange("p (dc l) -> p dc l", dc=DC)
            pas = a_psum_tiles()
            if cold:
                # first half kp-outer: matmuls start as soon as the first
                # chunk pairs land; second half ec-outer (all data resident
                # by then) so its casts stripe and don't trail into B(0)
                for kp in range(KP):
                    for ec in range(4):
                        a_drs(pas[ec], ec, kp, xh3, xl3, kp == 0, kp == KP - 1)
                for ec in range(4):
                    a_cast(pas[ec], ec, th8, tl8)
                for ec in range(4, DC):
                    for kp in range(KP):
                        a_drs(pas[ec], ec, kp, xh3, xl3, kp == 0, kp == KP - 1)
                    a_cast(pas[ec], ec, th8, tl8)
            else:
                for ec in range(DC):
                    for kp in range(KP):
                        a_drs(pas[ec], ec, kp, xh3, xl3, kp == 0, kp == KP - 1)
                    a_cast(pas[ec], ec, th8, tl8)
            return th8, tl8

        def step_b(oh, ol, th8, tl8, mid_cb=None):
            """[l, m] scores: lhsT = t chunks, rhs = io; then max + exp.
            mid_cb (the previous batch's transposes) is issued after the
            second lc group so its Act/DVE copies run before this batch's
            late exps/max clog those queues."""
            oh3 = oh[:].rearrange("p (et m) -> p et m", et=DC)
            ol3 = ol[:].rearrange("p (et m) -> p et m", et=DC)
            th3 = th8[:].rearrange("p (et l) -> p et l", et=DC)
            tl3 = tl8[:].rearrange("p (et l) -> p et l", et=DC)
            esbT = epool.tile([P, LC * L], F8, tag="esbT")
            ebias = small.tile([P, LC], F32, tag="ebias")
            for lc in range(LC):
                if lc == 3 and mid_cb is not None:
                    mid_cb()
                pb = mm_psum.tile([P, L], F32, tag="mm", name=f"pb{lc}")
                for kp in range(KP):
                    lh = th3[:, 2 * kp:2 * kp + 2, lc * P:(lc + 1) * P]
                    ll = tl3[:, 2 * kp:2 * kp + 2, lc * P:(lc + 1) * P]
                    rh = oh3[:, 2 * kp:2 * kp + 2, :]
                    rl = ol3[:, 2 * kp:2 * kp + 2, :]
                    nc.tensor.matmul(pb[:], lh, rh, start=(kp == 0),
                                     stop=False, perf_mode=DR)
                    nc.tensor.matmul(pb[:], ll, rh, start=False, stop=False,
                                     perf_mode=DR)
                    nc.tensor.matmul(pb[:], lh, rl, start=False,
                                     stop=(kp == KP - 1), perf_mode=DR)
                mx8 = small.tile([P, 8], F32, tag="mx8")
                nc.vector.max(mx8[:], pb[:])
                nc.vector.tensor_scalar_mul(ebias[:, lc:lc + 1], mx8[:, 0:1],
                                            -B_EXP)
                nc.scalar.activation(esbT[:, lc * L:(lc + 1) * L], pb[:], EXP,
                                     bias=ebias[:, lc:lc + 1], scale=B_EXP)
            return esbT

        def transpose_e(esbT, last=False):
            # transpose E[l, m] -> ET[m, l], packed 4 blocks per bank (from
            # the c pool — the mm pool's banks are still draining this
            # batch's exps); PSUM->SBUF copies alternate Act/DVE mid-run,
            # Pool/DVE for the last batch (Act then gates the tail exps+muls)
            esb_t = epool.tile([P, MC * L], F8, tag="esb_t")
            for mt in range(MC):
                tp = c_psum.tile([P, HD], F32, tag="pc", name=f"tp{mt}")
                tp8s = tp[:].bitcast(F8).rearrange("p (x two) -> p x two", two=2)
                for lc in range(LC):
                    nc.tensor.matmul(
                        tp8s[:, lc * P:(lc + 1) * P, 0:1],
                        esbT[:, lc * L + mt * P: lc * L + (mt + 1) * P],
                        ident[:], is_transpose=True)
                if mt % 2 == 0:
                    nc.scalar.activation(esb_t[:, mt * L:(mt + 1) * L],
                                         tp8s[:, 0:L, 0:1], COPY)
                else:
                    nc.vector.tensor_copy(esb_t[:, mt * L:(mt + 1) * L],
                                          tp8s[:, 0:L, 0:1])
            return esb_t

        def sums_c(b, esb_t, ih, il, wch, wcl, tail=False):
            et3 = esb_t[:].rearrange("p (mt l) -> p mt l", mt=MC)
            wh3 = wch[:].rearrange("p (mt o) -> p mt o", mt=MC)
            wl3 = wcl[:].rearrange("p (mt o) -> p mt o", mt=MC)
            ih3 = ih[:].rearrange("p (mt d) -> p mt d", mt=MC)
            il3 = il[:].rearrange("p (mt d) -> p mt d", mt=MC)

            def c_wave(lcs, q, pcs):
                for lc in lcs:
                    for half in range(2):
                        pc = pcs[(lc % 2) * 2 + half]
                        lhs = et3[:, 2 * q:2 * q + 2, lc * P:(lc + 1) * P]
                        rh = ih3[:, 2 * q:2 * q + 2, half * HD:(half + 1) * HD]
                        rl = il3[:, 2 * q:2 * q + 2, half * HD:(half + 1) * HD]
                        nc.tensor.matmul(pc[:], lhs, rh, start=(q == 0),
                                         stop=False, perf_mode=DR)
                        nc.tensor.matmul(pc[:], lhs, rl, start=False,
                                         stop=(q == MC // 2 - 1), perf_mode=DR)

            def sums_all(pss_all):
                # all 4 lc sums into ONE bank: sequential accumulation
                # groups at different columns, so only one mm slot is
                # needed (the first pb's, which frees earliest)
                for lc in range(LC):
                    for q in range(MC // 2):
                        lhs = et3[:, 2 * q:2 * q + 2, lc * P:(lc + 1) * P]
                        nc.tensor.matmul(pss_all[:, lc:lc + 1], lhs,
                                         wh3[:, 2 * q:2 * q + 2, :],
                                         start=(q == 0), stop=False,
                                         perf_mode=DR)
                        nc.tensor.matmul(pss_all[:, lc:lc + 1], lhs,
                                         wl3[:, 2 * q:2 * q + 2, :],
                                         start=False,
                                         stop=(q == MC // 2 - 1),
                                         perf_mode=DR)

            def norm_store(lcs, pcs, fine_last=False):
                if not fine_last:
                    # one combined [P, 2D] store for the whole two-lc wave:
                    # each dma_start costs ~0.5us of SP issue time, so fewer
                    # bigger stores keep the sequencer off the critical path
                    outt = outp.tile([P, 2 * D], F16, tag="outt2")
                    for i, lc in enumerate(lcs):
                        rc = rec[:, lc:lc + 1]
                        nc.scalar.activation(outt[:, i * D:i * D + HD],
                                             pcs[(lc % 2) * 2][:], COPY,
                                             scale=rc)
                        nc.vector.tensor_scalar_mul(
                            outt[:, i * D + HD:(i + 1) * D],
                            pcs[(lc % 2) * 2 + 1][:], rc)
                    nc.sync.dma_start(
                        out_d[b, lcs[0] * P:(lcs[-1] + 1) * P, :]
                        .rearrange("(lc p) d -> p lc d", p=P),
                        outt[:].rearrange("p (lc d) -> p lc d", lc=2))
                    return
                for lc in lcs:
                    outt = outp.tile([P, D], F16, tag="outt")
                    pc0 = pcs[(lc % 2) * 2]
                    pc1 = pcs[(lc % 2) * 2 + 1]
                    rc = rec[:, lc:lc + 1]
                    rows = slice(lc * P, (lc + 1) * P)
                    if fine_last and lc == lcs[-1]:
                        # quarter-width normalize chunks alternating Act/DVE;
                        # Act runs muls back-to-back (no DMA issues between
                        # them) and only issues the final store, the rest
                        # spread over SP/Pool so no sequencer serializes
                        QW = HD // 2
                        store_eng = [nc.sync, nc.gpsimd, nc.sync, nc.scalar]
                        for j in range(2):
                            cs = slice(j * QW, (j + 1) * QW)
                            c2 = slice(HD + j * QW, HD + (j + 1) * QW)
                            nc.scalar.activation(outt[:, cs], pc0[:, cs],
                                                 COPY, scale=rc)
                            nc.vector.tensor_scalar_mul(outt[:, c2],
                                                        pc1[:, cs], rc)
                            store_eng[2 * j].dma_start(out_d[b, rows, cs],
                                                       outt[:, cs])
                            store_eng[2 * j + 1].dma_start(
                                out_d[b, rows, c2], outt[:, c2])
                    else:
                        nc.scalar.activation(outt[:, 0:HD], pc0[:], COPY,
                                             scale=rc)
                        eng0 = nc.gpsimd if fine_last else nc.sync
                        eng0.dma_start(out_d[b, rows, 0:HD], outt[:, 0:HD])
                        nc.vector.tensor_scalar_mul(outt[:, HD:D], pc1[:], rc)
                        nc.sync.dma_start(out_d[b, rows, HD:D], outt[:, HD:D])

            # wave 1 (lc 0,1) from the c pool; sums interleave between the
            # q rounds so the PE never waits on ET copies or the reciprocal
            pcs1 = [c_psum.tile([P, HD], F32, tag="pc", name=f"pcw1_{j}")
                    for j in range(4)]
            pss_all = mm_psum.tile([P, LC], F32, tag="mm", name="pss")
            c_wave((0, 1), 0, pcs1)
            sums_all(pss_all)
            c_wave((0, 1), 1, pcs1)
            sums = small.tile([P, LC], F32, tag="sums")
            nc.vector.tensor_copy(sums[:], pss_all[:])
            rec = small.tile([P, LC], F32, tag="rec")
            nc.vector.reciprocal(rec[:], sums[:])
            # wave 2 (lc 2,3) from the mm pool (tp/pss slots freed by now)
            norm_store((0, 1), pcs1)
            pcs2 = [mm_psum.tile([P, HD], F32, tag="mm", name=f"pcw2_{j}")
                    for j in range(4)]
            c_wave((2, 3), 0, pcs2)
            c_wave((2, 3), 1, pcs2)
            norm_store((2, 3), pcs2, fine_last=tail)

        # ---- pipelined schedule:
        # A(b) | B(b).lc01 | T(b-1) | B(b).lc23 | sums+C(b-1) ----
        xh, xl = load_x_cold(0)
        rest = load_rest(0)
        prev = None
        for b in range(BL):
            th8, tl8 = step_a(xh, xl, cold=(b == 0))
            if b + 1 < BL:
                nxh, nxl = load_x(b + 1)
                nrest = load_rest(b + 1)
            oh, ol, ih, il, wch, wcl = rest
            holder = {}
            mid = None
            if prev is not None:
                def mid(pe=prev[1], holder=holder):
                    holder["et"] = transpose_e(pe)
            esbT = step_b(oh, ol, th8, tl8, mid_cb=mid)
            if prev is not None:
                sums_c(prev[0], holder["et"], *prev[2:])
            prev = (b, esbT, ih, il, wch, wcl)
            if b + 1 < BL:
                xh, xl, rest = nxh, nxl, nrest
        et = transpose_e(prev[1], last=True)
        sums_c(prev[0], et, *prev[2:], tail=True)

    nc.compile()
    return nc


def _host_prep(ix, iother, W, b):
    """Shard + fp8 hi/lo split prep on host."""
    M = np.ascontiguousarray(W.T) @ W                       # [D, D]
    u = W.T @ b                                             # [D]
    c = iother.reshape(-1, D) @ u                           # [B*L]
    wc = np.exp(c.reshape(B, L).astype(np.float32))         # e^c weights

    def split8(a, scale=1.0):
        a = a * scale if scale != 1.0 else a
        hi = a.astype(NF8)
        lo = (a - hi.astype(np.float32)).astype(NF8)
        return hi, lo

    mh, ml = split8(M, SM)
    ixt = np.ascontiguousarray(ix.transpose(0, 2, 1))       # [B, D, L]
    iot = np.ascontiguousarray(iother.transpose(0, 2, 1))   # [B, D, L]
    xh, xl = split8(ixt, SX)
    oh, ol = split8(iot, SO)
    iw = iother * wc[:, :, None]                            # e^c-weighted io
    ih, il = split8(iw)
    wh, wl = split8(wc)

    in_maps = []
    for core in range(NCORES):
        sl = slice(core * BL, (core + 1) * BL)
        in_maps.append({
            "xh": xh[sl], "xl": xl[sl],
            "oh": oh[sl], "ol": ol[sl],
            "ih": ih[sl], "il": il[sl],
            "mh": mh, "ml": ml,
            "wh": wh[sl], "wl": wl[sl],
        })
    return in_maps


def _get_nc():
    if "nc" not in _CACHE:
        _CACHE["nc"] = _build_program()
    return _CACHE["nc"]


def _get_runner():
    """Compile once; return (fn, in_names, out_names, out_shapes).

    Mirrors bass2jax.run_bass_via_pjrt's multi-core path but caches the
    jitted executable so repeated kernel() calls skip recompilation.
    """
    if "runner" in _CACHE:
        return _CACHE["runner"]
    import jax
    from jax.sharding import Mesh, PartitionSpec
    from jax.experimental.shard_map import shard_map
    from concourse import bass2jax
    from concourse import mybir as mb

    nc = _get_nc()
    bass2jax.install_neuronx_cc_hook()

    partition_name = (nc.partition_id_tensor.name
                      if nc.partition_id_tensor else None)
    in_names, out_names, out_avals, zero_shapes = [], [], [], []
    for alloc in nc.m.functions[0].allocations:
        if not isinstance(alloc, mb.MemoryLocationSet):
            continue
        name = alloc.memorylocations[0].name
        if alloc.kind == "ExternalInput":
            if name != partition_name:
                in_names.append(name)
        elif alloc.kind == "ExternalOutput":
            out_names.append(name)
            shape = tuple(alloc.tensor_shape)
            dtype = mb.dt.np(alloc.dtype)
            out_avals.append(jax.core.ShapedArray(shape, dtype))
            zero_shapes.append((shape, dtype))
    n_params = len(in_names)
    all_in_names = in_names + out_names
    if partition_name is not None:
        all_in_names = all_in_names + [partition_name]

    def _body(*args):
        operands = list(args)
        if partition_name is not None:
            operands.append(bass2jax.partition_id_tensor())
        outs = bass2jax._bass_exec_p.bind(
            *operands,
            out_avals=tuple(out_avals),
            in_names=tuple(all_in_names),
            out_names=tuple(out_names),
            lowering_input_output_aliases=(),
            sim_require_finite=True,
            sim_require_nnan=True,
            nc=nc,
        )
        return tuple(outs)

    devices = jax.devices()[:NCORES]
    mesh = Mesh(np.asarray(devices), ("core",))
    in_specs = (PartitionSpec("core"),) * (n_params + len(out_names))
    out_specs = (PartitionSpec("core"),) * len(out_names)
    donate = tuple(range(n_params, n_params + len(out_names)))
    fn = jax.jit(
        shard_map(_body, mesh=mesh, in_specs=in_specs, out_specs=out_specs,
                  check_rep=False),
        donate_argnums=donate, keep_unused=True)
    _CACHE["runner"] = (fn, in_names, out_names, zero_shapes)
    return _CACHE["runner"]


def _run(in_maps):
    fn, in_names, out_names, zero_shapes = _get_runner()
    concat_in = [
        np.concatenate([in_maps[c][name] for c in range(NCORES)], axis=0)
        for name in in_names
    ]
    zeros = [np.zeros((NCORES * s[0], *s[1:]), dt) for s, dt in zero_shapes]
    out_arrs = fn(*concat_in, *zeros)
    return {name: np.asarray(out_arrs[i]) for i, name in enumerate(out_names)}


def kernel(ix, iother, W, b):
    ix = np.asarray(ix, dtype=np.float32)
    iother = np.asarray(iother, dtype=np.float32)
    W = np.asarray(W, dtype=np.float32)
    b = np.asarray(b, dtype=np.float32)
    in_maps = _host_prep(ix, iother, W, b)
    # One retry: the device occasionally reports a transient
    # NRT_EXEC_UNIT_UNRECOVERABLE under rapid back-to-back runs.
    try:
        outs = _run(in_maps)
    except Exception:
        import time
        time.sleep(2.0)
        outs = _run(in_maps)
    return outs["out"].astype(np.float32)
